# revision 13
# baseline (speedup 1.0000x reference)
"""CirLinear Trainium2 kernel: y = x @ build_weight(W, alphas, gumbels)^T + bias.

Strategy (8 NeuronCores, no collectives), 2x4 grid:
 - core c = tshard*4 + oshard: tokens [8192*tshard, +8192), out cols
   [512*oshard, +512)
 - The circulant-averaged weight is built ON THE TENSOR ENGINE as
   selector-permutation matmuls, directly in transposed form W_eff^T
   (circulant averaging commutes with transpose), so the result lands in
   SBUF as the main matmul's lhsT with no DRAM roundtrip or transpose.
   For scale b: C_b(V) = Expand_b(Reduce_b(V)) where both passes are b
   permutation-selector matmuls ([128,128] stationary) with PSUM
   accumulation. Host packs V = W^T with out-columns bit-reversed and
   ic-interleaved so every residue class is a contiguous SBUF/PSUM block.
 - bf16 matmul with fp32 PSUM accumulation over 16 K-chunks, bias added
   on the scalar engine, bf16 output out^T [512, 8192] (host casts f32).
 - host assembles the 2x4 grid, un-bit-reverses rows, transposes back.
"""
import sys

sys.path.insert(0, '/opt/trn_rl_repo')

import numpy as np

import concourse.bass as bass
from concourse import bacc
import concourse.mybir as mybir
from concourse.tile import TileContext
from concourse.bass_utils import run_bass_kernel_spmd

N_CORES = 8
T_SHARDS, O_SHARDS = 2, 4
BATCH, TOKENS, IN_F, OUT_F = 16, 1024, 2048, 2048
TOK_TOTAL = BATCH * TOKENS            # 16384
TOK = TOK_TOTAL // T_SHARDS           # 8192 tokens per core
ROWS = OUT_F // O_SHARDS              # 512 out-features per core
SCALES = [2, 4, 8, 16, 32, 64]
N_IC = IN_F // 128                    # 16 contraction chunks
N_TG = TOK // 512                     # 16 token groups of 512
N_OS = ROWS // 128                    # 4 output-col subtiles / quarters
VW = N_IC * ROWS                      # 8192: width of v_sb / wT / acc space

# d_sb element offsets per scale (flat (c2, ic)-interleaved layout)
D_OFF = {}
_off = 0
for _b in SCALES:
    D_OFF[_b] = _off
    _off += VW // _b
D_TOT = _off                          # 8064

# selector slot bases (128-wide matrices): selr has identity at slot 0.
# The S_m selectors are symmetric (S_m[p,o]=1 iff p+o=m mod b within
# blocks), so the expand selectors E_j = S_j^T = S_j reuse the same slots.
SELR_BASE = {}
_s = 1
for _b in SCALES:
    SELR_BASE[_b] = _s
    _s += _b
SELR_SLOTS = _s                       # 127

bf16 = mybir.dt.bfloat16
f32 = mybir.dt.float32

_CACHE = {}


def _rev_bits(x, k):
    r = 0
    for _ in range(k):
        r = (r << 1) | (x & 1)
        x >>= 1
    return r


def _build_nc():
    nc = bacc.Bacc("TRN2", target_bir_lowering=False, debug=False, num_devices=N_CORES)
    xT = nc.dram_tensor("xT", [IN_F, TOK], bf16, kind="ExternalInput")
    v = nc.dram_tensor("v", [128, VW], bf16, kind="ExternalInput")
    selr = nc.dram_tensor("selr", [128, SELR_SLOTS * 128], bf16, kind="ExternalInput")
    bias_s = nc.dram_tensor("bias_s", [1, ROWS], f32, kind="ExternalInput")
    alphas = nc.dram_tensor("alphas", [1, 7], f32, kind="ExternalInput")
    gumbels = nc.dram_tensor("gumbels", [1, 7], f32, kind="ExternalInput")
    out = nc.dram_tensor("out", [ROWS, TOK], bf16, kind="ExternalOutput")

    with TileContext(nc) as tc:
        # ---------- softmax(alphas + gumbels) broadcast to 128 partitions ----------
        asb = nc.alloc_sbuf_tensor("asb", [128, 7], f32).ap()
        gsb = nc.alloc_sbuf_tensor("gsb", [128, 7], f32).ap()
        a_bc = nc.alloc_sbuf_tensor("a_bc", [128, 7], f32).ap()
        ssum = nc.alloc_sbuf_tensor("ssum", [128, 1], f32).ap()
        nc.gpsimd.dma_start(out=asb, in_=bass.AP(tensor=alphas, offset=0, ap=[[0, 128], [1, 7]]))
        nc.gpsimd.dma_start(out=gsb, in_=bass.AP(tensor=gumbels, offset=0, ap=[[0, 128], [1, 7]]))
        nc.vector.tensor_tensor(out=asb, in0=asb, in1=gsb, op=mybir.AluOpType.add)
        nc.scalar.activation(out=asb, in_=asb, func=mybir.ActivationFunctionType.Exp)
        nc.vector.tensor_reduce(out=ssum, in_=asb, axis=mybir.AxisListType.X, op=mybir.AluOpType.add)
        nc.vector.reciprocal(out=ssum, in_=ssum)
        nc.vector.tensor_scalar_mul(a_bc, asb, ssum)

        # ---------- bias: [1, 512] -> [128 part, 4] (per-osub per-partition) ----------
        bias_sb = nc.alloc_sbuf_tensor("bias_sb", [128, N_OS], f32).ap()
        with nc.allow_non_contiguous_dma(reason="512-element one-time bias transpose"):
            nc.gpsimd.dma_start(out=bias_sb, in_=bass.AP(tensor=bias_s, offset=0, ap=[[1, 128], [128, N_OS]]))

        # ---------- load packed V and selectors ----------
        v_sb = nc.alloc_sbuf_tensor("v_sb", [128, VW], bf16).ap()
        selr_sb = nc.alloc_sbuf_tensor("selr_sb", [128, SELR_SLOTS * 128], bf16).ap()
        d_sb = nc.alloc_sbuf_tensor("d_sb", [128, D_TOT], bf16).ap()
        wT = nc.alloc_sbuf_tensor("wT", [128, VW], bf16).ap()
        id_s = nc.alloc_sbuf_tensor("id_s", [128, 128], bf16).ap()

        # identity + scale-2 selectors first (tiny) so the reduce can start
        # as soon as v lands; v split across the two HWDGE queues; the two
        # biggest selector scales go on the SWDGE queue in parallel
        nc.sync.dma_start(out=selr_sb[:, 0:3 * 128], in_=selr.ap()[:, 0:3 * 128])
        nc.scalar.dma_start(out=v_sb[:, 0:VW // 2], in_=v.ap()[:, 0:VW // 2])
        nc.sync.dma_start(out=v_sb[:, VW // 2:], in_=v.ap()[:, VW // 2:])
        for b in (4, 8, 16):
            lo, hi = SELR_BASE[b] * 128, (SELR_BASE[b] + b) * 128
            nc.sync.dma_start(out=selr_sb[:, lo:hi], in_=selr.ap()[:, lo:hi])
        for b in (32, 64):
            lo, hi = SELR_BASE[b] * 128, (SELR_BASE[b] + b) * 128
            nc.gpsimd.dma_start(out=selr_sb[:, lo:hi], in_=selr.ap()[:, lo:hi])
        # a0-scaled identity (value a0 on the diagonal)
        nc.vector.tensor_scalar_mul(id_s, selr_sb[:, 0:128], a_bc[:, 0:1])

        with (
            tc.tile_pool(name="ps", bufs=2, space="PSUM") as ps_pool,
            tc.tile_pool(name="xt", bufs=3) as xt_pool,
            tc.tile_pool(name="osb", bufs=4) as osb_pool,
        ):
            # ---------- REDUCE: d_b = sum_m S_m^T @ V_block(m), scaled to d_sb ----------
            for idx, b in enumerate(SCALES, start=1):
                blk = VW // b                       # elements per residue block
                n_sub = (blk + 511) // 512          # 512-chunks per block
                n_t = (blk + 2047) // 2048          # psum tiles needed
                d_tiles = [ps_pool.tile([128, 2048], f32, name=f"d{b}_{t}", tag="ps")
                           for t in range(n_t)]
                cp = 0
                for s in range(n_sub):
                    fd = min(512, blk - s * 512)
                    ti, toff = divmod(s * 512, 2048)
                    dst_ps = d_tiles[ti][:, toff:toff + fd]
                    for q in range(b):
                        lhsT = selr_sb[:, (SELR_BASE[b] + q) * 128:(SELR_BASE[b] + q + 1) * 128]
                        rhs = v_sb[:, q * blk + s * 512: q * blk + s * 512 + fd]
                        nc.tensor.matmul(dst_ps, lhsT, rhs, start=(q == 0), stop=(q == b - 1))
                    # scale by a_b/b and store bf16 (alternate engines)
                    dst = d_sb[:, D_OFF[b] + s * 512: D_OFF[b] + s * 512 + fd]
                    nc.vector.tensor_scalar(out=dst, in0=dst_ps, scalar1=a_bc[:, idx:idx + 1],
                                            scalar2=1.0 / b, op0=mybir.AluOpType.mult,
                                            op1=mybir.AluOpType.mult)
                    cp += 1

            # ---------- EXPAND: acc = a0*V + sum_b sum_j E_j^T @ d_b, per quarter ----------
            for Q in range(N_OS):
                qt = ps_pool.tile([128, 2048], f32, name=f"acc{Q}", tag="ps")
                # gather the mm list: (out_off, fd, lhsT_lo, rhs_off) per bank
                mms = []      # (out_off, fd, sel_slot_lo, rhs_off)
                for idx, b in enumerate(SCALES, start=1):
                    blk = VW // b
                    qlo = (Q * 2048) // blk
                    qhi = ((Q + 1) * 2048 + blk - 1) // blk
                    for qj in range(qlo, qhi):
                        g0 = max(qj * blk, Q * 2048)
                        g1 = min((qj + 1) * blk, (Q + 1) * 2048)
                        g = g0
                        while g < g1:
                            fd = min(512, g1 - g)
                            mms.append((g - Q * 2048, fd,
                                        (SELR_BASE[b] + qj) * 128,
                                        D_OFF[b] + (g - qj * blk)))
                            g += fd
                # last writer per bank gets stop=True
                last_per_bank = {}
                for i, (ooff, fd, _, _) in enumerate(mms):
                    last_per_bank[ooff // 512] = i
                stop_idx = set(last_per_bank.values())
                # identity (a0) mms open each bank's accumulation group
                for t in range(4):
                    nc.tensor.matmul(qt[:, t * 512:(t + 1) * 512], id_s,
                                     v_sb[:, Q * 2048 + t * 512: Q * 2048 + (t + 1) * 512],
                                     start=True, stop=False, skip_group_check=True)
                for i, (ooff, fd, slo, roff) in enumerate(mms):
                    nc.tensor.matmul(qt[:, ooff:ooff + fd],
                                     selr_sb[:, slo:slo + 128],
                                     d_sb[:, roff:roff + fd],
                                     start=False, stop=(i in stop_idx),
                                     skip_group_check=True)
                # de-interleave quarter -> wT[:, ic*512 + Q*128 : +128]
                qt_r = qt.rearrange("p (c i) -> i p c", i=N_IC)
                for ic in range(N_IC):
                    dst = wT[:, ic * ROWS + Q * 128: ic * ROWS + Q * 128 + 128]
                    if ic % 2 == 0:
                        nc.vector.tensor_copy(out=dst, in_=qt_r[ic])
                    else:
                        nc.scalar.copy(out=dst, in_=qt_r[ic])

            # ---------- main matmul over 16 token groups ----------
            for tg in range(N_TG):
                xt = xt_pool.tile([128, N_IC * 512], bf16, name="xt")
                nc.sync.dma_start(
                    out=xt[:],
                    in_=bass.AP(tensor=xT, offset=tg * 512,
                                ap=[[TOK, 128], [128 * TOK, N_IC], [1, 512]]))
                pt = ps_pool.tile([128, 2048], f32, name=f"pt{tg}", tag="ps")
                for ic in range(N_IC):
                    rhs = xt[:, ic * 512:(ic + 1) * 512]
                    for o in range(N_OS):
                        nc.tensor.matmul(pt[:, o * 512:(o + 1) * 512],
                                         wT[:, ic * ROWS + o * 128: ic * ROWS + (o + 1) * 128],
                                         rhs, start=(ic == 0), stop=(ic == N_IC - 1))
                ot = osb_pool.tile([128, N_OS * 512], bf16, name="ot")
                for o in range(N_OS):
                    nc.scalar.activation(out=ot[:, o * 512:(o + 1) * 512],
                                         in_=pt[:, o * 512:(o + 1) * 512],
                                         func=mybir.ActivationFunctionType.Identity,
                                         bias=bias_sb[:, o:o + 1], scale=1.0)
                # one store per token group: rows (o,p), cols (t)
                nc.sync.dma_start(
                    out=bass.AP(tensor=out, offset=tg * 512,
                                ap=[[TOK, 128], [128 * TOK, N_OS], [1, 512]]),
                    in_=ot[:])

    nc.compile()
    return nc


def _selector_arrays():
    """Host constant: selr [128, 127*128] bf16 (identity + S_m per scale,
    in bit-reversed block-position order; S_m are symmetric so they also
    serve as the expand selectors)."""
    import ml_dtypes
    mats_r = [np.eye(128, dtype=np.float32)]
    for b in SCALES:
        k = int(np.log2(b))
        nb = 128 // b
        Ss = []
        for m in range(b):
            S = np.zeros((128, 128), dtype=np.float32)
            for pb in range(nb):
                for kk in range(b):
                    S[pb * b + (m - kk) % b, pb * b + kk] = 1.0
            Ss.append(S)
        for q in range(b):
            mats_r.append(Ss[_rev_bits(q, k)])
    selr = np.concatenate(mats_r, axis=1).astype(ml_dtypes.bfloat16)
    return np.ascontiguousarray(selr)


def make_in_maps(x, weight, bias, alphas, gumbels):
    import ml_dtypes
    x2 = np.asarray(x, np.float32).reshape(TOK_TOTAL, IN_F)
    xTh = np.ascontiguousarray(x2.T).astype(ml_dtypes.bfloat16)   # [2048, 16384]
    xslices = [np.ascontiguousarray(xTh[:, t * TOK:(t + 1) * TOK]) for t in range(T_SHARDS)]
    weight = np.asarray(weight, np.float32)
    bias = np.asarray(bias, np.float32)
    if "sel" not in _CACHE:
        _CACHE["sel"] = _selector_arrays()
    selr = _CACHE["sel"]
    rev9 = np.array([_rev_bits(c, 9) for c in range(ROWS)])
    vs, bs = [], []
    for o in range(O_SHARDS):
        V = weight[o * ROWS:(o + 1) * ROWS].T                      # [2048, 512]
        Vt = np.ascontiguousarray(V).reshape(N_IC, 128, ROWS)
        v_pack = Vt[:, :, rev9].transpose(1, 2, 0).reshape(128, VW)
        vs.append(np.ascontiguousarray(v_pack).astype(ml_dtypes.bfloat16))
        bs.append(np.ascontiguousarray(bias[o * ROWS:(o + 1) * ROWS][rev9]).reshape(1, ROWS))
    al = np.asarray(alphas, np.float32).reshape(1, 7)
    gu = np.asarray(gumbels, np.float32).reshape(1, 7)
    in_maps = []
    for c in range(N_CORES):
        t, o = divmod(c, O_SHARDS)
        in_maps.append({"xT": xslices[t], "v": vs[o], "selr": selr,
                        "bias_s": bs[o], "alphas": al, "gumbels": gu})
    return in_maps


def kernel(x, weight, bias, alphas, gumbels):
    if "nc" not in _CACHE:
        _CACHE["nc"] = _build_nc()
    nc = _CACHE["nc"]
    in_maps = make_in_maps(x, weight, bias, alphas, gumbels)
    res = run_bass_kernel_spmd(nc, in_maps, core_ids=list(range(N_CORES)))
    # assemble: rows = o-shards (bit-reversed within shard), cols = t-shards
    rev9 = np.array([_rev_bits(c, 9) for c in range(ROWS)])
    full_t = np.empty((OUT_F, TOK_TOTAL), dtype=np.float32)
    for o in range(O_SHARDS):
        blk = np.concatenate(
            [np.asarray(res.results[t * O_SHARDS + o]["out"], dtype=np.float32)
             for t in range(T_SHARDS)], axis=1)                    # [512, 16384]
        full_t[o * ROWS + rev9] = blk
    return np.ascontiguousarray(full_t.T).reshape(BATCH, TOKENS, OUT_F)
